# revision 1
# baseline (speedup 1.0000x reference)
"""Trainium2 Bass kernel for nn_Decoder (image-caption LSTM decoder w/ additive attention).

Sharding (8 cores):
- init matmuls (h0/c0): contraction-sharded (12288 dims/core) + AllReduce
- attention (e/scores/softmax): batch-sharded (8 rows/core)
- LSTM gates: hidden-sharded (64 hidden units -> 256 gate rows/core), needs
  per-step AllGather of unnormalized attention weights (exp(scores)) and h.
- ctx folded into gates via G = features @ W_ih[:, :ENC].T precompute
  (gates_ctx = sum_p alpha[n,p] * G[n,p,:], normalization deferred to after
  the matmul: gates = psum_ctx * (1/Z) + psum_eh).
- fc: vocab-sharded (1250 cols/core).

All heavy matmul operands in bf16 (PE streams 1 col/cycle regardless; fp32
would be 4x slower and doubles HBM traffic), fp32 PSUM accumulation.
"""

import sys

sys.path.insert(0, "/opt/trn_rl_repo")

import numpy as np
import ml_dtypes

import concourse.bass as bass
import concourse.bacc as bacc
import concourse.mybir as mybir
import concourse.tile as tile
from concourse.bass_utils import run_bass_kernel_spmd

F32 = mybir.dt.float32
BF16 = mybir.dt.bfloat16
FP8 = mybir.dt.float8e4
AX = mybir.AxisListType
OP = mybir.AluOpType
ACT = mybir.ActivationFunctionType

N, P, ENC = 64, 64, 1536
T = 32
V, E, A, H = 10000, 512, 512, 512
NC_ = 8          # cores
NB = N // NC_    # 8  batch rows per core (attention shard)
HS = H // NC_    # 64 hidden units per core
GS = 4 * HS      # 256 gate rows per core
VS = V // NC_    # 1250 vocab cols per core
KS = P * ENC // NC_  # 12288 init contraction per core
CT = ENC // 128  # 12 contraction tiles over ENC
NPT = N * P // 128  # 32 np tiles

G_DT = BF16      # dtype for the G tensor stream (BF16 or FP8)

bf16 = ml_dtypes.bfloat16

_cache = {}


def _build():
    nc = bacc.Bacc(None, target_bir_lowering=False, debug=False)

    # ---- I/O ----
    featT = nc.dram_tensor("featT", [128, CT, N * P], BF16, kind="ExternalInput")
    featT_loc = nc.dram_tensor("featT_loc", [128, CT, NB * P], BF16, kind="ExternalInput")
    feat_initT = nc.dram_tensor("feat_initT", [128, 96, 64], BF16, kind="ExternalInput")
    initW = nc.dram_tensor("initW", [128, 96, 1024], BF16, kind="ExternalInput")
    w_ehT = nc.dram_tensor("w_ehT", [128, 8, GS], BF16, kind="ExternalInput")
    w_ihcT = nc.dram_tensor("w_ihcT", [128, CT, GS], BF16, kind="ExternalInput")
    b_eh = nc.dram_tensor("b_eh", [1, GS], F32, kind="ExternalInput")
    tok_WT = nc.dram_tensor("tok_WT", [128, 4, A], BF16, kind="ExternalInput")
    feat_WT = nc.dram_tensor("feat_WT", [128, CT, A], BF16, kind="ExternalInput")
    b_attn = nc.dram_tensor("b_attn", [128, 4], F32, kind="ExternalInput")
    w_full = nc.dram_tensor("w_full", [128, 4], BF16, kind="ExternalInput")
    embTd = nc.dram_tensor("embT", [128, 4, T, N], BF16, kind="ExternalInput")
    fc_WTd = nc.dram_tensor("fc_WT", [128, 4, VS], BF16, kind="ExternalInput")
    fc_bd = nc.dram_tensor("fc_b", [1, VS], F32, kind="ExternalInput")
    ident = nc.dram_tensor("ident", [128, 128], F32, kind="ExternalInput")
    preds_out = nc.dram_tensor("preds", [T * N, VS], F32, kind="ExternalOutput")

    rank = nc.partition_id()

    with tile.TileContext(nc) as tc:
        with (
            tc.tile_pool(name="persist", bufs=1) as pp,
            tc.tile_pool(name="dram", bufs=2, space="DRAM") as dram,
        ):
            # ---------------- persistent small tensors ----------------
            tokW = pp.tile([128, 4, A], BF16)
            nc.sync.dma_start(tokW[:], tok_WT[:])
            featW = pp.tile([128, CT, A], BF16)
            nc.sync.dma_start(featW[:], feat_WT[:])
            battn = pp.tile([128, 4], F32)
            nc.sync.dma_start(battn[:], b_attn[:])
            wfull = pp.tile([128, 4], BF16)
            nc.sync.dma_start(wfull[:], w_full[:])
            wehT = pp.tile([128, 8, GS], BF16)
            nc.sync.dma_start(wehT[:], w_ehT[:])
            beh1 = pp.tile([1, GS], F32)
            nc.sync.dma_start(beh1[:], b_eh[:])
            beh_rep = pp.tile([128, GS], F32)
            nc.gpsimd.partition_broadcast(beh_rep[:], beh1[:])
            idn = pp.tile([128, 128], F32)
            nc.sync.dma_start(idn[:], ident[:])
            idn16 = pp.tile([128, 128], BF16)
            nc.vector.tensor_copy(idn16[:], idn[:])
            finitT = pp.tile([128, 96, 64], BF16)
            nc.sync.dma_start(finitT[:], feat_initT[:])
            attn_img = pp.tile([128, 4, NB * P], BF16)  # [a_lo, a_hi, (n,p)]
            G_sb = pp.tile([128, NPT, GS], G_DT)        # [(j,p), pair, g]
            # transposed h0/c0 partials for the AllReduce:
            # [:, 0:4, :] = h0T [h_lo, h_hi, n]; [0:64, 4:12, :] = c0T [c_lo, k, n]
            hc_sb = pp.tile([128, 12, 64], F32)
            nc.vector.memset(hc_sb[64:128, 4:12, :], 0.0)

            # ---------------- phase 1+2: attn_img and G (uses featT) ----------------
            with (
                tc.tile_pool(name="feat", bufs=1) as pf,
                tc.tile_pool(name="psum_pre", bufs=1, space="PSUM") as psp,
            ):
                ftl = pf.tile([128, CT, NB * P], BF16)
                nc.sync.dma_start(ftl[:], featT_loc[:])
                ft = pf.tile([128, CT, N * P], BF16)
                nc.sync.dma_start(ft[:], featT[:])
                wihc = pf.tile([128, CT, GS], BF16)
                nc.sync.dma_start(wihc[:], w_ihcT[:])

                # attn_img: [a, np_local] = feat_W.T @ featT_loc  (+ bias)
                ps_ai = psp.tile([128, 4, NB * P], F32)
                for q in range(4):
                    for c in range(CT):
                        nc.tensor.matmul(
                            ps_ai[:, q, :],
                            featW[:, c, 128 * q : 128 * (q + 1)],
                            ftl[:, c, :],
                            start=(c == 0),
                            stop=(c == CT - 1),
                        )
                    nc.scalar.activation(
                        attn_img[:, q, :], ps_ai[:, q, :], ACT.Identity,
                        bias=battn[:, q : q + 1],
                    )

                # G: per np-tile m: [128(np), GS] = featT_tile.T @ w_ihcT (+ b_eh)
                with tc.tile_pool(name="psum_g", bufs=2, space="PSUM") as psg:
                    for m in range(NPT):
                        ps_g = psg.tile([128, GS], F32, tag="g")
                        for c in range(CT):
                            nc.tensor.matmul(
                                ps_g[:],
                                ft[:, c, 128 * m : 128 * (m + 1)],
                                wihc[:, c, :],
                                start=(c == 0),
                                stop=(c == CT - 1),
                            )
                        nc.vector.scalar_tensor_tensor(
                            G_sb[:, m, :], ps_g[:], 1.0, beh_rep[:],
                            op0=OP.mult, op1=OP.add,
                        )

            # ---------------- phase 3: init h0/c0 ----------------
            NCH = 8  # k-tiles per DMA chunk
            with (
                tc.tile_pool(name="initw", bufs=3) as piw,
                tc.tile_pool(name="psum_i", bufs=1, space="PSUM") as psi,
            ):
                ps_h0 = psi.tile([64, 512], F32)
                ps_c0 = psi.tile([64, 512], F32)
                for ch in range(96 // NCH):
                    iw = piw.tile([128, NCH, 1024], BF16, tag="iw")
                    nc.sync.dma_start(iw[:], initW[:, ch * NCH : (ch + 1) * NCH, :])
                    for k in range(NCH):
                        kt = ch * NCH + k
                        nc.tensor.matmul(
                            ps_h0[:], finitT[:, kt, :], iw[:, k, 0:512],
                            start=(kt == 0), stop=(kt == 95),
                        )
                        nc.tensor.matmul(
                            ps_c0[:], finitT[:, kt, :], iw[:, k, 512:1024],
                            start=(kt == 0), stop=(kt == 95),
                        )
                # evacuate + transpose h0 -> [h, n] and c0 -> [c_lo, k, n]
                h_ev = piw.tile([64, 512], F32)
                nc.vector.tensor_copy(h_ev[:], ps_h0[:])
                c_ev = piw.tile([64, 512], F32)
                nc.vector.tensor_copy(c_ev[:], ps_c0[:])
                with tc.tile_pool(name="psum_t0", bufs=1, space="PSUM") as pst:
                    ps_ht = pst.tile([128, 4, 64], F32)
                    for q in range(4):
                        nc.tensor.transpose(
                            ps_ht[:, q, :], h_ev[:, 128 * q : 128 * (q + 1)],
                            idn[0:64, 0:64],
                        )
                    nc.vector.tensor_copy(hc_sb[:, 0:4, :], ps_ht[:])
                    ps_ct = pst.tile([64, 8, 64], F32)
                    for k8 in range(8):
                        nc.tensor.transpose(
                            ps_ct[:, k8, :], c_ev[:, 64 * k8 : 64 * (k8 + 1)],
                            idn[0:64, 0:64],
                        )
                    nc.vector.tensor_copy(hc_sb[0:64, 4:12, :], ps_ct[:])
            hc_in = dram.tile([128, 12, 64], F32)
            hc_out = dram.tile([128, 12, 64], F32)
            nc.sync.dma_start(hc_in[:], hc_sb[:])
            nc.gpsimd.collective_compute(
                "AllReduce", OP.add, replica_groups=[list(range(NC_))],
                ins=[hc_in[:]], outs=[hc_out[:]],
            )

            # ---------------- loop-persistent state ----------------
            with (
                tc.tile_pool(name="lp", bufs=1) as lp,
            ):
                emb = lp.tile([128, 4, T, N], BF16)
                nc.sync.dma_start(emb[:], embTd[:])
                fcW = lp.tile([128, 4, VS], BF16)
                nc.sync.dma_start(fcW[:], fc_WTd[:])
                fcb1 = lp.tile([1, VS], F32)
                nc.sync.dma_start(fcb1[:], fc_bd[:])
                fcb_rep = lp.tile([128, VS], F32)
                nc.gpsimd.partition_broadcast(fcb_rep[:], fcb1[:])
                h_T = lp.tile([128, 4, N], BF16)       # [h_lo, h_hi, n] full h
                h_loc = lp.tile([128, 4, NB], BF16)    # local batch rows of h
                h_allT = lp.tile([128, 4, T, N], BF16)  # h_1..h_T for fc
                c_a = lp.tile([64, HS], F32)
                c_b = lp.tile([64, HS], F32)
                alpha_m = lp.tile([128, N], BF16)      # masked exp(scores) pairs
                nc.vector.memset(alpha_m[:], 0.0)

                # h0 -> h_T / h_loc (f32 staging + cast), c0 slice -> c_a
                st = lp.tile([128, 4, N], F32)
                nc.sync.dma_start(st[:], hc_out[:, 0:4, :])
                nc.vector.tensor_copy(h_T[:], st[:])
                stl = lp.tile([128, 4, NB], F32)
                for hh in range(4):
                    nc.sync.dma_start(
                        stl[:, hh, :], hc_out[:, hh, bass.ts(rank, NB)]
                    )
                nc.vector.tensor_copy(h_loc[:], stl[:])
                # c0 slice arrives transposed [c_lo, n]; PE-transpose back
                c_t0 = lp.tile([64, 64], F32)
                nc.sync.dma_start(
                    c_t0[:],
                    hc_out[0:64, 4:12, :][:, bass.DynSlice(rank, 1), :].rearrange(
                        "c one n -> c (one n)"
                    ),
                )
                with tc.tile_pool(name="psum_c0t", bufs=1, space="PSUM") as psc:
                    ps_c0t = psc.tile([64, 64], F32)
                    nc.tensor.transpose(ps_c0t[:], c_t0[:], idn[0:64, 0:64])
                    nc.vector.tensor_copy(c_a[:], ps_c0t[:])

                # ---------------- phase 4: recurrence ----------------
                with (
                    tc.tile_pool(name="loop", bufs=2) as pl,
                    tc.tile_pool(name="psum_l", bufs=1, space="PSUM") as psl,
                    tc.tile_pool(name="dram_l", bufs=2, space="DRAM") as drl,
                ):
                    c_cur, c_nxt = c_a, c_b
                    for t in range(T):
                        # attn_h for local rows: [a, 4, NB]
                        ps_ah = psl.tile([128, 4, NB], F32, tag="ah")
                        for q in range(4):
                            for kt in range(4):
                                nc.tensor.matmul(
                                    ps_ah[:, q, :],
                                    tokW[:, kt, 128 * q : 128 * (q + 1)],
                                    h_loc[:, kt, :],
                                    start=(kt == 0), stop=(kt == 3),
                                )
                        ah = pl.tile([128, 4, NB], BF16, tag="ah_sb")
                        nc.vector.tensor_copy(ah[:], ps_ah[:])

                        # e = relu(attn_img + attn_h bcast); scores = w . e
                        ps_sc = psl.tile([1, NB * P], F32, tag="sc")
                        for q in range(4):
                            epre = pl.tile([128, NB, P], BF16, tag="epre")
                            i0 = attn_img[:, q, :].rearrange("a (n p) -> a n p", p=P)
                            i1 = ah[:, q, :].rearrange("a (n one) -> a n one", one=1)
                            i0b, i1b = bass.broadcast_tensor_aps(i0, i1)
                            nc.vector.tensor_tensor(epre[:], i0b, i1b, op=OP.add)
                            e_t = pl.tile([128, NB * P], BF16, tag="e")
                            nc.vector.tensor_scalar_max(
                                e_t[:], epre[:].rearrange("a n p -> a (n p)"), 0.0
                            )
                            nc.tensor.matmul(
                                ps_sc[:], wfull[:, q : q + 1], e_t[:],
                                start=(q == 0), stop=(q == 3),
                            )
                        exps = pl.tile([1, NB * P], BF16, tag="exps")
                        nc.scalar.activation(exps[:], ps_sc[:], ACT.Exp)

                        # exchange exp(scores)
                        ab_in = drl.tile([1, NB * P], BF16, tag="abi")
                        ab_out = drl.tile([NC_, NB * P], BF16, tag="abo")
                        nc.sync.dma_start(ab_in[:], exps[:])
                        nc.gpsimd.collective_compute(
                            "AllGather", OP.bypass,
                            replica_groups=[list(range(NC_))],
                            ins=[ab_in[:]], outs=[ab_out[:]],
                        )
                        # alpha_m[(j,p), 2m+j] = exps[n=2m+j, p]
                        av = ab_out[:].rearrange("c (l p) -> (c l) p", p=P)  # [n, p]
                        for j in range(2):
                            nc.sync.dma_start(
                                alpha_m[64 * j : 64 * (j + 1), :].rearrange(
                                    "p (m two) -> p m two", two=2
                                )[:, :, j],
                                av.rearrange("(m j) p -> j p m", j=2)[j],
                            )
                        # Z = sum_p exps -> reciprocal  [64n, 1]
                        z_sb = pl.tile([64, P], BF16, tag="z")
                        nc.sync.dma_start(z_sb[:], av)
                        z1 = pl.tile([64, 1], F32, tag="z1")
                        nc.vector.tensor_reduce(z1[:], z_sb[:], AX.X, OP.add)
                        rz = pl.tile([64, 1], F32, tag="rz")
                        nc.vector.reciprocal(rz[:], z1[:])

                        # gates: ctx part (pair-packed G) and eh part, [g, n]
                        ps_cx = psl.tile([128, 2, N], F32, tag="cx")
                        ps_eh = psl.tile([128, 2, N], F32, tag="eh")
                        for gt in range(2):
                            for kt in range(8):
                                rhs = emb[:, kt, t, :] if kt < 4 else h_T[:, kt - 4, :]
                                nc.tensor.matmul(
                                    ps_eh[:, gt, :],
                                    wehT[:, kt, 128 * gt : 128 * (gt + 1)],
                                    rhs,
                                    start=(kt == 0), stop=(kt == 7),
                                )
                            for m in range(NPT):
                                nc.tensor.matmul(
                                    ps_cx[:, gt, 2 * m : 2 * m + 2],
                                    G_sb[:, m, 128 * gt : 128 * (gt + 1)],
                                    alpha_m[:, 2 * m : 2 * m + 2],
                                    start=(m == 0), stop=(m == NPT - 1),
                                )
                        # transpose both to [n, g] and combine with 1/Z scaling
                        cx_sb = pl.tile([128, 2, N], F32, tag="cx_sb")
                        nc.vector.tensor_copy(cx_sb[:], ps_cx[:])
                        eh_sb = pl.tile([128, 2, N], F32, tag="eh_sb")
                        nc.vector.tensor_copy(eh_sb[:], ps_eh[:])
                        ps_t = psl.tile([64, 4, 128], F32, tag="pt")
                        for gt in range(2):
                            nc.tensor.transpose(
                                ps_t[:, gt, :], cx_sb[:, gt, :], idn[:]
                            )
                            nc.tensor.transpose(
                                ps_t[:, 2 + gt, :], eh_sb[:, gt, :], idn[:]
                            )
                        gates = pl.tile([64, GS], F32, tag="gates")
                        eh_t = pl.tile([64, 2, 128], F32, tag="eh_t")
                        nc.vector.tensor_copy(eh_t[:], ps_t[:, 2:4, :])
                        for gt in range(2):
                            nc.vector.scalar_tensor_tensor(
                                gates[:, 128 * gt : 128 * (gt + 1)],
                                ps_t[:, gt, :], rz[:], eh_t[:, gt, :],
                                op0=OP.mult, op1=OP.add,
                            )

                        # pointwise LSTM (gate order i|f then g|o)
                        sif = pl.tile([64, 128], F32, tag="sif")
                        nc.scalar.activation(sif[:], gates[:, 0:128], ACT.Sigmoid)
                        tg = pl.tile([64, HS], F32, tag="tg")
                        nc.scalar.activation(tg[:], gates[:, 128:192], ACT.Tanh)
                        so = pl.tile([64, HS], F32, tag="so")
                        nc.scalar.activation(so[:], gates[:, 192:256], ACT.Sigmoid)
                        ig = pl.tile([64, HS], F32, tag="ig")
                        nc.vector.tensor_tensor(ig[:], sif[:, 0:HS], tg[:], op=OP.mult)
                        fc_ = pl.tile([64, HS], F32, tag="fc_")
                        nc.vector.tensor_tensor(
                            fc_[:], sif[:, HS:128], c_cur[:], op=OP.mult
                        )
                        nc.vector.tensor_tensor(c_nxt[:], ig[:], fc_[:], op=OP.add)
                        tcl = pl.tile([64, HS], F32, tag="tc")
                        nc.scalar.activation(tcl[:], c_nxt[:], ACT.Tanh)
                        hnew = pl.tile([64, HS], BF16, tag="hnew")
                        nc.vector.tensor_tensor(hnew[:], so[:], tcl[:], op=OP.mult)
                        c_cur, c_nxt = c_nxt, c_cur
                        # transpose h_new -> [h_lo, n] so the post-AG layout is clean
                        ps_ht2 = psl.tile([64, HS], BF16, tag="ht2")
                        nc.tensor.transpose(ps_ht2[:], hnew[:], idn16[0:64, 0:64])
                        hnewT = pl.tile([64, HS], BF16, tag="hnewT")
                        nc.vector.tensor_copy(hnewT[:], ps_ht2[:])

                        # h exchange (blocks are [h_lo, n])
                        hb_in = drl.tile([64, HS], BF16, tag="hbi")
                        hb_out = drl.tile([NC_, 64, HS], BF16, tag="hbo")
                        nc.sync.dma_start(hb_in[:], hnewT[:])
                        nc.gpsimd.collective_compute(
                            "AllGather", OP.bypass,
                            replica_groups=[list(range(NC_))],
                            ins=[hb_in[:]], outs=[hb_out[:]],
                        )
                        for q in range(4):
                            nc.sync.dma_start(
                                h_T[:, q, :],
                                hb_out[2 * q : 2 * q + 2, :, :].rearrange(
                                    "c hl n -> (c hl) n"
                                ),
                            )
                            nc.sync.dma_start(
                                h_loc[:, q, :],
                                hb_out[2 * q : 2 * q + 2, :, bass.ts(rank, NB)]
                                .rearrange("c hl n -> (c hl) n"),
                            )
                        nc.vector.tensor_copy(h_allT[:, :, t, :], h_T[:])
                        if t % 2 == 1:
                            j = t // 2
                            for lo, hi in [(0, 512), (512, 1024), (1024, VS)]:
                                ps_f = psl.tile([128, 512], F32, tag="f")
                                for kt in range(4):
                                    nc.tensor.matmul(
                                        ps_f[:, 0 : hi - lo],
                                        h_allT[:, kt, 2 * j : 2 * j + 2, :],
                                        fcW[:, kt, lo:hi],
                                        start=(kt == 0), stop=(kt == 3),
                                    )
                                ob = pl.tile([128, 512], F32, tag="ob")
                                nc.vector.scalar_tensor_tensor(
                                    ob[:, 0 : hi - lo], ps_f[:, 0 : hi - lo], 1.0,
                                    fcb_rep[:, lo:hi], op0=OP.mult, op1=OP.add,
                                )
                                nc.sync.dma_start(
                                    preds_out[128 * j : 128 * (j + 1), lo:hi],
                                    ob[:, 0 : hi - lo],
                                )


    nc.compile()
    return nc


def _prep(inputs):
    """Host-side layout prep (slicing/transposes/dtype casts only)."""
    f = np.ascontiguousarray(np.asarray(inputs["features"], np.float32))
    cap = np.asarray(inputs["captions"])
    embd_W = np.asarray(inputs["embd_W"], np.float32)
    tokW = np.asarray(inputs["attn_token_W"], np.float32)
    tokb = np.asarray(inputs["attn_token_b"], np.float32)
    featW = np.asarray(inputs["attn_feat_W"], np.float32)
    featb = np.asarray(inputs["attn_feat_b"], np.float32)
    wfull = np.asarray(inputs["attn_full_W"], np.float32)
    W_ih = np.asarray(inputs["W_ih"], np.float32)
    b_ih = np.asarray(inputs["b_ih"], np.float32)
    W_hh = np.asarray(inputs["W_hh"], np.float32)
    b_hh = np.asarray(inputs["b_hh"], np.float32)
    fc_W = np.asarray(inputs["fc_W"], np.float32)
    fc_b = np.asarray(inputs["fc_b"], np.float32)
    iWh = np.asarray(inputs["init_Wh"], np.float32)
    iWc = np.asarray(inputs["init_Wc"], np.float32)

    def t128(x, pdim):  # (pdim*128, rest...) -> [128, pdim, rest]
        return np.ascontiguousarray(
            x.reshape(pdim, 128, *x.shape[1:]).transpose(1, 0, *range(2, x.ndim + 1))
        )

    featT_full = t128(f.reshape(N * P, ENC).T.copy(), CT).astype(bf16)  # [128,12,4096]
    # emb: (N,T,E) -> (E,T,N) -> [128,4,T,N]
    embT = t128(embd_W[cap].transpose(2, 1, 0).reshape(E, T * N), 4)
    embT = embT.reshape(128, 4, T, N).astype(bf16)
    tok_WT = t128(tokW.T.copy(), 4).astype(bf16)        # [128,4,512]
    feat_WT = t128(featW.T.copy(), CT).astype(bf16)     # [128,12,512]
    b_attn = np.ascontiguousarray((tokb + featb).reshape(4, 128).T).astype(np.float32)
    w_full8 = np.ascontiguousarray(wfull[0].reshape(4, 128).T).astype(bf16)
    ident = np.eye(128, dtype=np.float32)

    in_maps = []
    for k in range(NC_):
        u = np.arange(HS * k, HS * (k + 1))
        rows = np.concatenate([u, H + u, 2 * H + u, 3 * H + u])
        w_eh = np.concatenate([W_ih[rows][:, ENC:], W_hh[rows]], axis=1)  # (256,1024)
        w_ehT = t128(w_eh.T.copy(), 8).astype(bf16)
        w_ihcT = t128(W_ih[rows][:, :ENC].T.copy(), CT).astype(bf16)
        b_eh = (b_ih[rows] + b_hh[rows]).reshape(1, GS).astype(np.float32)
        # attention shard: batch rows [NB*k, NB*(k+1))
        fbat = np.ascontiguousarray(f[NB * k : NB * (k + 1)])  # (8, 64, 1536)
        featT_loc = t128(fbat.reshape(NB * P, ENC).T.copy(), CT).astype(bf16)
        # init shard: pixel slice [NB*k, NB*(k+1)) across all batch rows
        fpix = np.ascontiguousarray(f[:, NB * k : NB * (k + 1), :])  # (64, 8, 1536)
        feat_initT = t128(fpix.reshape(N, KS).T.copy(), 96).astype(bf16)
        initW = t128(
            np.concatenate(
                [iWh[KS * k : KS * (k + 1)], iWc[KS * k : KS * (k + 1)]], axis=1
            ), 96
        ).astype(bf16)
        vsl = slice(VS * k, VS * (k + 1))
        fc_WT = t128(fc_W[vsl].T.copy(), 4).astype(bf16)
        in_maps.append({
            "featT": featT_full, "featT_loc": featT_loc,
            "feat_initT": feat_initT, "initW": initW,
            "w_ehT": w_ehT, "w_ihcT": w_ihcT, "b_eh": b_eh,
            "tok_WT": tok_WT, "feat_WT": feat_WT, "b_attn": b_attn,
            "w_full": w_full8, "embT": embT,
            "fc_WT": fc_WT, "fc_b": fc_b[vsl].reshape(1, VS).astype(np.float32),
            "ident": ident,
        })
    return in_maps


def kernel(**inputs) -> np.ndarray:
    if "nc" not in _cache:
        _cache["nc"] = _build()
    nc = _cache["nc"]
    in_maps = _prep(inputs)
    res = run_bass_kernel_spmd(nc, in_maps, core_ids=list(range(NC_)), trace=False)
    parts = [
        r["preds"].reshape(T, N, VS).transpose(1, 0, 2) for r in res.results
    ]
    return np.ascontiguousarray(np.concatenate(parts, axis=2))



# revision 12
# speedup vs baseline: 5.7205x; 5.7205x over previous
"""Trainium2 Bass kernel for nn_Decoder (image-caption LSTM decoder w/ additive attention).

Sharding (8 cores): pure data-parallel over batch N (8 rows/core, weights
replicated) -> ZERO per-step collectives. Only the h0/c0 init matmul is
contraction-sharded (12288 dims/core) with a single ReduceScatter.

Key structure per core:
- preamble: attn_img = featW^T @ featT_loc (+bias);
            G[(p,n), g] = featT_loc^T @ W_ih[:, :ENC]^T  (ctx folded into gates);
            gates_emb[g, t, n] = W_ih[:, ENC:]^T @ emb (+ b_ih + b_hh), all T at once;
            h0/c0 partials [n, hc] (k-sharded) -> ReduceScatter -> transpose.
            Init matmuls are emission-interleaved with G so PE consumes initW
            DMA chunks as they arrive.
- per step: attn_h (PE) -> e = relu(attn_img + ah) (DVE bf16) -> scoresT via
            e-as-stationary matmuls (PE, out [128=(p,n), 1]) -> exp (Act) ->
            alpha_m = exp * mask8 (DVE) -> ctx-gates = G^T @ alpha_m + Z row
            via ones^T @ alpha_m (PE) -> gates = ps_cx * (1/Z) + ps_eh + gemb_t
            (DVE) -> LSTM pointwise. Sigmoid computed as 1/(1+exp(-x)) so every
            activation (exp/tanh/identity/copy) lives in ONE act table set --
            no 1.28us table reloads in the loop. h stays [h, n]: no transposes.
- fc: full vocab per core (local 8 rows), in t-halves: half 0 interleaved over
  steps 16..31, half 1 in the tail; evacs alternate DVE/Act; bf16 DRAM out,
  f32 upcast on host.

Gate order is host-reordered to (i, f, o, g).
"""

import sys

sys.path.insert(0, "/opt/trn_rl_repo")

import numpy as np
import ml_dtypes

import concourse.bass as bass
import concourse.bacc as bacc
import concourse.mybir as mybir
import concourse.tile as tile
from concourse.bass_utils import run_bass_kernel_spmd

F32 = mybir.dt.float32
BF16 = mybir.dt.bfloat16
AX = mybir.AxisListType
OP = mybir.AluOpType
ACT = mybir.ActivationFunctionType

N, P, ENC = 64, 64, 1536
T = 32
V, E, A, H = 10000, 512, 512, 512
NC_ = 8            # cores
NB = N // NC_      # 8 local batch rows
G4 = 4 * H         # 2048 gate rows
GT = G4 // 128     # 16 gate tiles
CT = ENC // 128    # 12 enc contraction tiles
KS = P * ENC // NC_  # 12288 init contraction per core
KT = KS // 128     # 96 init k-tiles
VP = 10112         # padded vocab (79 * 128)
VT = VP // 128     # 79 vocab tiles

bf16 = ml_dtypes.bfloat16

_cache = {}


def _build():
    nc = bacc.Bacc(None, target_bir_lowering=False, debug=False)

    # ---- I/O (per-core layouts prepared on host) ----
    featWT = nc.dram_tensor("featWT", [128, CT, A], BF16, kind="ExternalInput")
    featT_loc = nc.dram_tensor("featT_loc", [128, CT, NB * P], BF16, kind="ExternalInput")
    b_attn = nc.dram_tensor("b_attn", [128, 4], F32, kind="ExternalInput")
    tok_WT = nc.dram_tensor("tok_WT", [128, 4, A], BF16, kind="ExternalInput")
    w_full = nc.dram_tensor("w_full", [128, 4], BF16, kind="ExternalInput")
    w_ihcT = nc.dram_tensor("w_ihcT", [128, CT, G4], BF16, kind="ExternalInput")
    w_ieT = nc.dram_tensor("w_ieT", [128, 4, G4], BF16, kind="ExternalInput")
    w_hhT = nc.dram_tensor("w_hhT", [128, 4, G4], BF16, kind="ExternalInput")
    b_eh = nc.dram_tensor("b_eh", [128, GT], F32, kind="ExternalInput")
    embT = nc.dram_tensor("embT", [128, 4, T, NB], BF16, kind="ExternalInput")
    fc_WT = nc.dram_tensor("fc_WT", [128, 4, VP], BF16, kind="ExternalInput")
    fc_b = nc.dram_tensor("fc_b", [128, VT], F32, kind="ExternalInput")
    feat_initT = nc.dram_tensor("feat_initT", [128, KT, N], BF16, kind="ExternalInput")
    initW = nc.dram_tensor("initW", [128, KT, 1024], BF16, kind="ExternalInput")
    mask8 = nc.dram_tensor("mask8", [128, NB], BF16, kind="ExternalInput")
    ones128 = nc.dram_tensor("ones128", [128, 1], BF16, kind="ExternalInput")
    ones_all = nc.dram_tensor("ones_all", [128, 128], BF16, kind="ExternalInput")
    ident16 = nc.dram_tensor("ident16", [128, 128], BF16, kind="ExternalInput")
    preds_out = nc.dram_tensor("preds", [VP, T, NB], BF16, kind="ExternalOutput")

    NPF = NB * P        # 512 flat (p, n), p-major
    NCH = 4             # initW k-tiles per DMA chunk
    NCHUNKS = KT // NCH
    VG = 20             # fc vocab tiles per output DMA group

    with tile.TileContext(nc) as tc:
        with (
            tc.tile_pool(name="persist", bufs=1) as pp,
            tc.tile_pool(name="dram", bufs=1, space="DRAM") as dram,
        ):
            # ---------------- persistent tiles ----------------
            tokW = pp.tile([128, 4, A], BF16)
            wfull = pp.tile([128, 4], BF16)
            whh = pp.tile([128, 4, G4], BF16)
            emb = pp.tile([128, 4, T, NB], BF16)
            msk = pp.tile([128, NB], BF16)
            ones = pp.tile([128, 1], BF16)
            onesA = pp.tile([128, 128], BF16)
            idn16 = pp.tile([128, 128], BF16)
            attn_img = pp.tile([128, 4, NPF], BF16)     # [a, q, (p, n)]
            G_sb = pp.tile([128, 4, G4], BF16)          # [(p16, n), chunk, g]
            gemb = pp.tile([128, GT, T * NB], BF16)     # [g, tile, (t, n)]
            h_hist = pp.tile([128, 4, T, NB], BF16)     # [h, kt, t, n]
            h0_sb = pp.tile([128, 4, NB], BF16)
            c_a = pp.tile([128, 4, NB], F32)
            c_b = pp.tile([128, 4, NB], F32)
            beh = pp.tile([128, GT], F32)

            # small-weight DMAs first (cheap), then initW stream, then the
            # big G/emb weights; fcW is deferred via pool-address reuse.
            nc.sync.dma_start(tokW[:], tok_WT[:])
            nc.sync.dma_start(wfull[:], w_full[:])
            nc.sync.dma_start(msk[:], mask8[:])
            nc.sync.dma_start(ones[:], ones128[:])
            nc.sync.dma_start(onesA[:], ones_all[:])
            nc.sync.dma_start(idn16[:], ident16[:])
            nc.sync.dma_start(beh[:], b_eh[:])
            nc.sync.dma_start(emb[:], embT[:])

            with (
                tc.tile_pool(name="pre", bufs=1) as pf,
                tc.tile_pool(name="initw", bufs=5) as piw,
                tc.tile_pool(name="psum_pre", bufs=2, space="PSUM") as psp,
                tc.tile_pool(name="psum_init", bufs=1, space="PSUM") as psi,
            ):
                fW = pf.tile([128, CT, A], BF16)
                nc.sync.dma_start(fW[:], featWT[:])
                ftl = pf.tile([128, CT, NPF], BF16)
                nc.sync.dma_start(ftl[:], featT_loc[:])
                battn = pf.tile([128, 4], F32)
                nc.sync.dma_start(battn[:], b_attn[:])
                wihc = pf.tile([128, CT, G4], BF16)
                nc.sync.dma_start(wihc[:], w_ihcT[:])
                finitT = pf.tile([128, KT, N], BF16)
                nc.sync.dma_start(finitT[:], feat_initT[:])

                # initW stream (the DMA long pole): chunk DMAs issued before
                # the remaining gate weights so h0/c0 lands ASAP.
                iw_tiles = []
                for ch in range(NCHUNKS):
                    iw = piw.tile([128, NCH, 1024], BF16, tag="iw", name=f"iw{ch}")
                    nc.sync.dma_start(iw[:], initW[:, ch * NCH : (ch + 1) * NCH, :])
                    iw_tiles.append(iw)
                wie = pf.tile([128, 4, G4], BF16)
                for c4 in range(4):
                    nc.sync.dma_start(wie[:, c4, :], w_ieT[:, c4, :])
                for c4 in range(4):
                    nc.sync.dma_start(whh[:, c4, :], w_hhT[:, c4, :])

                # PE work, emission-interleaved so init consumes DMA chunks
                # while G/attn_img/gemb fill the gaps.
                ps_hc = psi.tile([64, 1024], F32)

                def init_chunk(ch):
                    iw = iw_tiles[ch]
                    for k in range(NCH):
                        kt = ch * NCH + k
                        for hf in range(2):
                            nc.tensor.matmul(
                                ps_hc[:, 512 * hf : 512 * (hf + 1)],
                                finitT[:, kt, 0:64],
                                iw[:, k, 512 * hf : 512 * (hf + 1)],
                                start=(kt == 0), stop=(kt == KT - 1),
                            )

                def attn_img_q(q):
                    ps_ai = psp.tile([128, NPF], F32, tag="pre", name=f"ai{q}")
                    for c in range(CT):
                        nc.tensor.matmul(
                            ps_ai[:], fW[:, c, 128 * q : 128 * (q + 1)],
                            ftl[:, c, :], start=(c == 0), stop=(c == CT - 1),
                        )
                    nc.scalar.activation(
                        attn_img[:, q, :], ps_ai[:], ACT.Identity,
                        bias=battn[:, q : q + 1],
                    )

                def g_piece(ch, gp):
                    ps_g = psp.tile([128, 512], F32, tag="pre", name=f"g{ch}_{gp}")
                    for c in range(CT):
                        nc.tensor.matmul(
                            ps_g[:],
                            ftl[:, c, 128 * ch : 128 * (ch + 1)],
                            wihc[:, c, 512 * gp : 512 * (gp + 1)],
                            start=(c == 0), stop=(c == CT - 1),
                        )
                    if gp % 2 == 0:
                        nc.vector.tensor_copy(
                            G_sb[:, ch, 512 * gp : 512 * (gp + 1)], ps_g[:]
                        )
                    else:
                        nc.scalar.activation(
                            G_sb[:, ch, 512 * gp : 512 * (gp + 1)], ps_g[:], ACT.Copy,
                        )

                def gemb_piece(gp):
                    ps_e = psp.tile([128, 2, T * NB], F32, tag="pre", name=f"ge{gp}")
                    for g2 in range(2):
                        k = 2 * gp + g2
                        for c in range(4):
                            nc.tensor.matmul(
                                ps_e[:, g2, :],
                                wie[:, c, 128 * k : 128 * (k + 1)],
                                emb[:, c, :, :].rearrange("e t n -> e (t n)"),
                                start=(c == 0), stop=(c == 3),
                            )
                    b0, b1 = bass.broadcast_tensor_aps(
                        ps_e[:],
                        beh[:, 2 * gp : 2 * gp + 2].rearrange("g (k o) -> g k o", o=1),
                    )
                    nc.vector.tensor_tensor(
                        gemb[:, 2 * gp : 2 * gp + 2, :], b0, b1, op=OP.add
                    )

                # emission order matches expected DMA arrival: attn_img
                # (feat weights land first), a few G pieces, then init chunks
                # alternating with the rest of G; gemb last (wie lands after
                # the initW stream).
                for q in range(4):
                    attn_img_q(q)
                gl = [(ch, gp) for ch in range(4) for gp in range(4)]
                for i in range(3):
                    g_piece(*gl[i])
                gi = 3
                for ch in range(NCHUNKS):
                    init_chunk(ch)
                    if gi < len(gl):
                        g_piece(*gl[gi])
                        gi += 1
                while gi < len(gl):
                    g_piece(*gl[gi])
                    gi += 1
                for gp in range(8):
                    gemb_piece(gp)

                # evac partials, exchange, transpose to [hc, n]
                hc_sb = pf.tile([64, 1024], F32)
                nc.vector.tensor_copy(hc_sb[:], ps_hc[:])
                hcp_d = dram.tile([64, 1024], F32)
                hcl_d = dram.tile([NB, 1024], F32)
                nc.scalar.dma_start(hcp_d[:], hc_sb[:])
                nc.gpsimd.collective_compute(
                    "ReduceScatter", OP.add, replica_groups=[list(range(NC_))],
                    ins=[hcp_d[:]], outs=[hcl_d[:]],
                )
                hcl = pf.tile([NB, 1024], F32)
                nc.scalar.dma_start(hcl[:], hcl_d[:])
                hcl16 = pf.tile([NB, 1024], BF16)
                nc.vector.tensor_copy(hcl16[:], hcl[:])
                with tc.tile_pool(name="psum_t", bufs=1, space="PSUM") as pst_p:
                    ps_t = pst_p.tile([128, 8, NB], BF16)
                    for j in range(8):
                        nc.tensor.transpose(
                            ps_t[:, j, :], hcl16[:, 128 * j : 128 * (j + 1)],
                            idn16[0:NB, 0:NB],
                        )
                    nc.vector.tensor_copy(h0_sb[:], ps_t[:, 0:4, :])
                    nc.vector.tensor_copy(c_a[:], ps_t[:, 4:8, :])

            # ---------------- recurrence + interleaved fc ----------------
            with (
                tc.tile_pool(name="fcw", bufs=1) as pfc,
                tc.tile_pool(name="loop", bufs=2) as pl,
                tc.tile_pool(name="stage", bufs=2) as pst,
                tc.tile_pool(name="ps_ah", bufs=1, space="PSUM") as ps_ah_p,
                tc.tile_pool(name="ps_st", bufs=1, space="PSUM") as ps_st_p,
                tc.tile_pool(name="ps_z", bufs=1, space="PSUM") as ps_z_p,
                tc.tile_pool(name="ps_cx", bufs=1, space="PSUM") as ps_cx_p,
                tc.tile_pool(name="ps_eh", bufs=1, space="PSUM") as ps_eh_p,
                tc.tile_pool(name="ps_fc", bufs=3, space="PSUM") as ps_fc_p,
            ):
                fcW = pfc.tile([128, 4, VP], BF16)
                nc.sync.dma_start(fcW[:], fc_WT[:])
                fcb = pfc.tile([128, VT], F32)
                nc.sync.dma_start(fcb[:], fc_b[:])

                c_cur, c_nxt = c_a, c_b
                fc_stage = {}
                fc_ei = [0]
                tail_mode = [False]

                def emit_fc(vt, half):
                    ps_f = ps_fc_p.tile([128, 16 * NB], F32, tag="f")
                    for c in range(4):
                        nc.tensor.matmul(
                            ps_f[:],
                            fcW[:, c, 128 * vt : 128 * (vt + 1)],
                            h_hist[:, c, 16 * half : 16 * (half + 1), :].rearrange(
                                "h t n -> h (t n)"
                            ),
                            start=(c == 0), stop=(c == 3),
                        )
                    g0 = (vt // VG) * VG
                    if vt == g0:
                        fc_stage[(g0, half)] = pst.tile(
                            [128, VG, 16 * NB], BF16, tag="st", name=f"st{g0}_{half}"
                        )
                    st = fc_stage[(g0, half)]
                    if tail_mode[0] and fc_ei[0] % 2 == 0:
                        f0, f1 = bass.broadcast_tensor_aps(
                            ps_f[:], fcb[:, vt : vt + 1]
                        )
                        nc.vector.tensor_tensor(st[:, vt - g0, :], f0, f1, op=OP.add)
                    else:
                        nc.scalar.activation(
                            st[:, vt - g0, :], ps_f[:], ACT.Identity,
                            bias=fcb[:, vt : vt + 1],
                        )
                    fc_ei[0] += 1
                    if vt == min(g0 + VG, VT) - 1:
                        gn = vt - g0 + 1
                        nc.sync.dma_start(
                            preds_out[
                                128 * g0 : 128 * (g0 + gn),
                                16 * half : 16 * (half + 1), :,
                            ].rearrange("(g v) t n -> v g (t n)", g=gn),
                            st[:, 0:gn, :],
                        )

                for t in range(T):
                    h_prev = h0_sb[:, :, :] if t == 0 else h_hist[:, :, t - 1, :]

                    # attn_h: [a-tile, q, n]
                    ps_ah = ps_ah_p.tile([128, 4, NB], F32, tag="ah")
                    for q in range(4):
                        for kt in range(4):
                            nc.tensor.matmul(
                                ps_ah[:, q, :],
                                tokW[:, kt, 128 * q : 128 * (q + 1)],
                                h_prev[:, kt, :],
                                start=(kt == 0), stop=(kt == 3),
                            )

                    # W_hh gates (ready at step start; fills PE while DVE works)
                    ps_eh = ps_eh_p.tile([128, GT, NB], F32, tag="eh")
                    for k in range(GT):
                        for c in range(4):
                            nc.tensor.matmul(
                                ps_eh[:, k, :],
                                whh[:, c, 128 * k : 128 * (k + 1)],
                                h_prev[:, c, :],
                                start=(c == 0), stop=(c == 3),
                            )

                    # veh = ps_eh + gemb_t does not depend on the alpha path;
                    # hoist it off the critical chain
                    v_eh = pl.tile([128, GT, NB], F32, tag="veh")
                    nc.vector.tensor_tensor(
                        v_eh[:], ps_eh[:], gemb[:, :, NB * t : NB * (t + 1)], op=OP.add
                    )

                    # interleaved fc (half 0 over steps 16..31)
                    if t >= 16:
                        lo = (VT * (t - 16)) // 16
                        hi = (VT * (t - 15)) // 16
                        for vt in range(lo, hi):
                            emit_fc(vt, 0)

                    # e = relu(attn_img + ah), free order (q, p, n)
                    ah = pl.tile([128, 4, NB], BF16, tag="ahsb")
                    nc.vector.tensor_copy(ah[:], ps_ah[:])
                    e_pre = pl.tile([128, 4, P, NB], BF16, tag="epre")
                    i0 = attn_img[:, :, :].rearrange("a q (p n) -> a q p n", n=NB)
                    i1 = ah[:, :, :].rearrange("a q (o n) -> a q o n", o=1)
                    i0b, i1b = bass.broadcast_tensor_aps(i0, i1)
                    nc.vector.tensor_tensor(e_pre[:], i0b, i1b, op=OP.add)
                    e_sb = pl.tile([128, 4, NPF], BF16, tag="e")
                    nc.vector.tensor_scalar_max(
                        e_sb[:], e_pre[:].rearrange("a q p n -> a q (p n)"), 0.0
                    )

                    # scoresT: [(p16, n) chunk, c] = e_chunk^T @ w_full
                    ps_st = ps_st_p.tile([128, 4, 1], F32, tag="sT")
                    for ch in range(4):
                        for q in range(4):
                            nc.tensor.matmul(
                                ps_st[:, ch, :],
                                e_sb[:, q, 128 * ch : 128 * (ch + 1)],
                                wfull[:, q : q + 1],
                                start=(q == 0), stop=(q == 3),
                            )
                    exps = pl.tile([128, 4, 1], BF16, tag="exps")
                    nc.scalar.activation(exps[:], ps_st[:], ACT.Exp)

                    # alpha_m[(p16, n), ch, n'] = exp * mask
                    alpha_m = pl.tile([128, 4, NB], BF16, tag="alm")
                    am0, am1 = bass.broadcast_tensor_aps(
                        exps[:, :, :], msk[:].rearrange("r (o n) -> r o n", o=1)
                    )
                    nc.vector.tensor_tensor(alpha_m[:], am0, am1, op=OP.mult)

                    # Z row first (its result chain gates the evac); the
                    # all-ones stationary replicates Z across all partitions,
                    # so reciprocal lands pre-broadcast -- no gpsimd hop.
                    ps_z = ps_z_p.tile([128, NB], F32, tag="z")
                    for ch in range(4):
                        nc.tensor.matmul(
                            ps_z[:], onesA[:], alpha_m[:, ch, :],
                            start=(ch == 0), stop=(ch == 3),
                        )
                    ps_cx = ps_cx_p.tile([128, GT, NB], F32, tag="cx")
                    for k in range(GT):
                        for ch in range(4):
                            nc.tensor.matmul(
                                ps_cx[:, k, :],
                                G_sb[:, ch, 128 * k : 128 * (k + 1)],
                                alpha_m[:, ch, :],
                                start=(ch == 0), stop=(ch == 3),
                            )
                    rz_rep = pl.tile([128, NB], F32, tag="rzr")
                    nc.vector.reciprocal(rz_rep[:], ps_z[:])

                    # gates = ps_cx * rz + veh
                    t0 = pl.tile([128, GT, NB], F32, tag="t0")
                    c0b, c1b = bass.broadcast_tensor_aps(
                        ps_cx[:, :, :], rz_rep[:].rearrange("g (o n) -> g o n", o=1)
                    )
                    nc.vector.tensor_tensor(t0[:], c0b, c1b, op=OP.mult)
                    gates = pl.tile([128, GT, NB], F32, tag="gates")
                    nc.vector.tensor_tensor(gates[:], t0[:], v_eh[:], op=OP.add)

                    # pointwise LSTM; tiles: i 0:4, f 4:8, o 8:12, g 12:16.
                    # One tanh(x/2) covers sigmoid gates AND tanh(g) (g-gate
                    # weights are host-doubled). State conventions: h~ = 2h
                    # (tokW/whh/fcW host-halved), c~ = 2c (init_Wc doubled).
                    #   sg_f = 0.5 th_f + 0.5
                    #   u = sg_f * c~          (= 2 sigmoid(f) c)
                    #   v = (th_i + 1) * th_g  (= 2 sigmoid(i) tanh(g))
                    #   c~_new = u + v
                    #   tanh_c = tanh(0.5 c~_new)
                    #   h~ = (th_o + 1) * tanh_c
                    th = pl.tile([128, GT, NB], F32, tag="th")
                    nc.scalar.activation(th[:], gates[:], ACT.Tanh, scale=0.5)
                    u2 = pl.tile([128, 4, NB], F32, tag="u2")
                    nc.vector.scalar_tensor_tensor(
                        u2[:], th[:, 4:8, :], 1.0, c_cur[:], op0=OP.add, op1=OP.mult,
                    )
                    t2 = pl.tile([128, 4, NB], F32, tag="t2")
                    nc.vector.scalar_tensor_tensor(
                        t2[:], th[:, 0:4, :], 1.0, th[:, 12:16, :],
                        op0=OP.add, op1=OP.mult,
                    )
                    nc.vector.scalar_tensor_tensor(
                        c_nxt[:], u2[:], 0.5, t2[:], op0=OP.mult, op1=OP.add,
                    )
                    tc_t = pl.tile([128, 4, NB], F32, tag="tc")
                    nc.scalar.activation(tc_t[:], c_nxt[:], ACT.Tanh, scale=0.5)
                    nc.vector.scalar_tensor_tensor(
                        h_hist[:, :, t, :], th[:, 8:12, :], 1.0, tc_t[:],
                        op0=OP.add, op1=OP.mult,
                    )
                    c_cur, c_nxt = c_nxt, c_cur

                # tail: fc half 1 (evacs alternate DVE/Act; both idle now)
                tail_mode[0] = True
                for vt in range(VT):
                    emit_fc(vt, 1)

    nc.compile()
    return nc


def _prep(inputs):
    """Host-side layout prep (slicing/transposes/dtype casts only)."""
    f = np.ascontiguousarray(np.asarray(inputs["features"], np.float32))
    cap = np.asarray(inputs["captions"])
    embd_W = np.asarray(inputs["embd_W"], np.float32)
    tokW = np.asarray(inputs["attn_token_W"], np.float32)
    tokb = np.asarray(inputs["attn_token_b"], np.float32)
    featW = np.asarray(inputs["attn_feat_W"], np.float32)
    featb = np.asarray(inputs["attn_feat_b"], np.float32)
    wfull = np.asarray(inputs["attn_full_W"], np.float32)
    W_ih = np.asarray(inputs["W_ih"], np.float32)
    b_ih = np.asarray(inputs["b_ih"], np.float32)
    W_hh = np.asarray(inputs["W_hh"], np.float32)
    b_hh = np.asarray(inputs["b_hh"], np.float32)
    fc_W = np.asarray(inputs["fc_W"], np.float32)
    fc_b = np.asarray(inputs["fc_b"], np.float32)
    iWh = np.asarray(inputs["init_Wh"], np.float32)
    iWc = np.asarray(inputs["init_Wc"], np.float32)

    def t128(x, pdim):  # (pdim*128, rest...) -> [128, pdim, rest]
        return np.ascontiguousarray(
            x.reshape(pdim, 128, *x.shape[1:]).transpose(1, 0, *range(2, x.ndim + 1))
        )

    # gate reorder (i, f, g, o) -> (i, f, o, g)
    ro = np.concatenate([
        np.arange(0, H), np.arange(H, 2 * H),
        np.arange(3 * H, 4 * H), np.arange(2 * H, 3 * H),
    ])

    featWT = t128(featW.T.copy(), CT).astype(bf16)              # [128,12,512]
    # h is carried as 2h on device; c as 2c; the g-gate rows are doubled so a
    # single tanh(x/2) computes both sigmoid halves and tanh(g).
    gsc = np.concatenate([np.ones(3 * H), 2.0 * np.ones(H)]).astype(np.float32)
    tok_WT = t128(0.5 * tokW.T.copy(), 4).astype(bf16)          # [128,4,512]
    b_attn = np.ascontiguousarray((tokb + featb).reshape(4, 128).T).astype(np.float32)
    w_full8 = np.ascontiguousarray(wfull[0].reshape(4, 128).T).astype(bf16)
    w_ihcT = t128((W_ih[ro] * gsc[:, None])[:, :ENC].T.copy(), CT).astype(bf16)
    w_ieT = t128((W_ih[ro] * gsc[:, None])[:, ENC:].T.copy(), 4).astype(bf16)
    w_hhT = t128((W_hh[ro] * (0.5 * gsc)[:, None]).T.copy(), 4).astype(bf16)
    b_eh = np.ascontiguousarray(
        ((b_ih[ro] + b_hh[ro]) * gsc).reshape(GT, 128).T
    ).astype(np.float32)                                        # [128,16]
    fc_pad = np.zeros((VP, H), np.float32)
    fc_pad[:V] = 0.5 * fc_W
    fc_WT = t128(fc_pad.T.copy(), 4).astype(bf16)               # [128,4,10112]
    fcb_pad = np.zeros((VP,), np.float32)
    fcb_pad[:V] = fc_b
    fc_b8 = np.ascontiguousarray(fcb_pad.reshape(VT, 128).T).astype(np.float32)
    mask8 = (np.arange(128)[:, None] % NB == np.arange(NB)[None, :]).astype(bf16)
    ones128 = np.ones((128, 1), bf16)
    ones_all = np.ones((128, 128), bf16)
    ident16 = np.eye(128).astype(bf16)

    in_maps = []
    for k in range(NC_):
        fl = f[NB * k : NB * (k + 1)]                    # (8, 64, 1536)
        featT_loc = t128(
            np.ascontiguousarray(fl.transpose(2, 1, 0)).reshape(ENC, P * NB), CT
        ).astype(bf16)                                   # [128,12,512] (p-major)
        embT = t128(
            np.ascontiguousarray(
                embd_W[cap[NB * k : NB * (k + 1)]].transpose(2, 1, 0)
            ).reshape(E, T * NB), 4,
        ).reshape(128, 4, T, NB).astype(bf16)
        fpix = np.ascontiguousarray(f[:, NB * k : NB * (k + 1), :])  # (64, 8, 1536)
        feat_initT = t128(fpix.reshape(N, KS).T.copy(), KT).astype(bf16)
        initWc = t128(
            2.0 * np.concatenate(
                [iWh[KS * k : KS * (k + 1)], iWc[KS * k : KS * (k + 1)]], axis=1
            ), KT,
        ).astype(bf16)                                   # [128,96,1024] (2h, 2c)
        in_maps.append({
            "featWT": featWT, "featT_loc": featT_loc, "b_attn": b_attn,
            "tok_WT": tok_WT, "w_full": w_full8,
            "w_ihcT": w_ihcT, "w_ieT": w_ieT, "w_hhT": w_hhT, "b_eh": b_eh,
            "embT": embT, "fc_WT": fc_WT, "fc_b": fc_b8,
            "feat_initT": feat_initT, "initW": initWc,
            "mask8": mask8, "ones128": ones128, "ones_all": ones_all,
            "ident16": ident16,
        })
    return in_maps


def kernel(**inputs) -> np.ndarray:
    if "nc" not in _cache:
        _cache["nc"] = _build()
    nc = _cache["nc"]
    in_maps = _prep(inputs)
    res = run_bass_kernel_spmd(nc, in_maps, core_ids=list(range(NC_)), trace=False)
    parts = [
        np.asarray(r["preds"][:V]).astype(np.float32).transpose(2, 1, 0)
        for r in res.results
    ]
    return np.ascontiguousarray(np.concatenate(parts, axis=0))


if __name__ == "__main__":
    import time
    t0 = time.time()
    nc = _build()
    print(f"build+compile: {time.time()-t0:.1f}s")
    from concourse.timeline_sim import TimelineSim
    ts = TimelineSim(nc, trace=False)
    print(f"TimelineSim: {ts.simulate():.0f} ns")


# revision 15
# speedup vs baseline: 5.9157x; 1.0341x over previous
"""Trainium2 Bass kernel for nn_Decoder (image-caption LSTM decoder w/ additive attention).

Sharding (8 cores): pure data-parallel over batch N (8 rows/core, weights
replicated) -> ZERO per-step collectives. Only the h0/c0 init matmul is
contraction-sharded (12288 dims/core) with a single ReduceScatter.

Key structure per core:
- preamble: attn_img = featW^T @ featT_loc (+bias);
            G[(p,n), g] = featT_loc^T @ W_ih[:, :ENC]^T  (ctx folded into gates);
            gates_emb[g, t, n] = W_ih[:, ENC:]^T @ emb (+ b_ih + b_hh), all T at once;
            h0/c0 partials [n, hc] (k-sharded) -> ReduceScatter -> transpose.
            Init matmuls are emission-interleaved with G so PE consumes initW
            DMA chunks as they arrive.
- per step: attn_h (PE) -> e = relu(attn_img + ah) (DVE bf16) -> scoresT via
            e-as-stationary matmuls (PE, out [128=(p,n), 1]) -> exp (Act) ->
            alpha_m = exp * mask8 (DVE) -> ctx-gates = G^T @ alpha_m + Z row
            via ones^T @ alpha_m (PE) -> gates = ps_cx * (1/Z) + ps_eh + gemb_t
            (DVE) -> LSTM pointwise. Sigmoid computed as 1/(1+exp(-x)) so every
            activation (exp/tanh/identity/copy) lives in ONE act table set --
            no 1.28us table reloads in the loop. h stays [h, n]: no transposes.
- fc: full vocab per core (local 8 rows), in t-halves: half 0 interleaved over
  steps 16..31, half 1 in the tail; evacs alternate DVE/Act; bf16 DRAM out,
  f32 upcast on host.

Gate order is host-reordered to (i, f, o, g).
"""

import sys

sys.path.insert(0, "/opt/trn_rl_repo")

import numpy as np
import ml_dtypes

import concourse.bass as bass
import concourse.bacc as bacc
import concourse.mybir as mybir
import concourse.tile as tile
from concourse.bass_utils import run_bass_kernel_spmd

F32 = mybir.dt.float32
BF16 = mybir.dt.bfloat16
AX = mybir.AxisListType
OP = mybir.AluOpType
ACT = mybir.ActivationFunctionType

N, P, ENC = 64, 64, 1536
T = 32
V, E, A, H = 10000, 512, 512, 512
NC_ = 8            # cores
NB = N // NC_      # 8 local batch rows
G4 = 4 * H         # 2048 gate rows
GT = G4 // 128     # 16 gate tiles
CT = ENC // 128    # 12 enc contraction tiles
KS = P * ENC // NC_  # 12288 init contraction per core
KT = KS // 128     # 96 init k-tiles
VP = 10112         # padded vocab (79 * 128)
VT = VP // 128     # 79 vocab tiles

bf16 = ml_dtypes.bfloat16

_cache = {}


def _build():
    nc = bacc.Bacc(None, target_bir_lowering=False, debug=False)

    # ---- I/O (per-core layouts prepared on host) ----
    featWT = nc.dram_tensor("featWT", [128, CT, A], BF16, kind="ExternalInput")
    featT_loc = nc.dram_tensor("featT_loc", [128, CT, NB * P], BF16, kind="ExternalInput")
    b_attn = nc.dram_tensor("b_attn", [128, 4], F32, kind="ExternalInput")
    tok_WT = nc.dram_tensor("tok_WT", [128, 4, A], BF16, kind="ExternalInput")
    w_full = nc.dram_tensor("w_full", [128, 4], BF16, kind="ExternalInput")
    w_ihcT = nc.dram_tensor("w_ihcT", [128, CT, G4], BF16, kind="ExternalInput")
    w_ieT = nc.dram_tensor("w_ieT", [128, 4, G4], BF16, kind="ExternalInput")
    w_hhT = nc.dram_tensor("w_hhT", [128, 4, G4], BF16, kind="ExternalInput")
    b_eh = nc.dram_tensor("b_eh", [128, GT], F32, kind="ExternalInput")
    embT = nc.dram_tensor("embT", [128, 4, T, NB], BF16, kind="ExternalInput")
    fc_WT = nc.dram_tensor("fc_WT", [128, 4, VP], BF16, kind="ExternalInput")
    fc_b = nc.dram_tensor("fc_b", [128, VT], F32, kind="ExternalInput")
    feat_initT = nc.dram_tensor("feat_initT", [128, KT, N], BF16, kind="ExternalInput")
    initW = nc.dram_tensor("initW", [128, KT, 1024], BF16, kind="ExternalInput")
    mask8 = nc.dram_tensor("mask8", [128, NB], BF16, kind="ExternalInput")
    ones128 = nc.dram_tensor("ones128", [128, 1], BF16, kind="ExternalInput")
    ones_all = nc.dram_tensor("ones_all", [128, 128], BF16, kind="ExternalInput")
    ident16 = nc.dram_tensor("ident16", [128, 128], BF16, kind="ExternalInput")
    preds_out = nc.dram_tensor("preds", [VP, T, NB], BF16, kind="ExternalOutput")

    NPF = NB * P        # 512 flat (p, n), p-major
    NCH = 4             # initW k-tiles per DMA chunk
    NCHUNKS = KT // NCH
    VG = 20             # fc vocab tiles per output DMA group

    with tile.TileContext(nc) as tc:
        with (
            tc.tile_pool(name="persist", bufs=1) as pp,
            tc.tile_pool(name="dram", bufs=1, space="DRAM") as dram,
        ):
            # ---------------- persistent tiles ----------------
            tokW = pp.tile([128, 4, A], BF16)
            wfull = pp.tile([128, 4], BF16)
            whh = pp.tile([128, 4, G4], BF16)
            emb = pp.tile([128, 4, T, NB], BF16)
            msk = pp.tile([128, NB], BF16)
            ones = pp.tile([128, 1], BF16)
            onesA = pp.tile([128, 128], BF16)
            idn16 = pp.tile([128, 128], BF16)
            attn_img = pp.tile([128, 4, NPF], BF16)     # [a, q, (p, n)]
            G_sb = pp.tile([128, 4, G4], BF16)          # [(p16, n), chunk, g]
            gemb = pp.tile([128, GT, T * NB], BF16)     # [g, tile, (t, n)]
            h_hist = pp.tile([128, 4, T, NB], BF16)     # [h, kt, t, n]
            h0_sb = pp.tile([128, 4, NB], BF16)
            c_a = pp.tile([128, 4, NB], F32)
            c_b = pp.tile([128, 4, NB], F32)
            beh = pp.tile([128, GT], F32)

            # small-weight DMAs first (cheap), then initW stream, then the
            # big G/emb weights; fcW is deferred via pool-address reuse.
            nc.sync.dma_start(tokW[:], tok_WT[:])
            nc.sync.dma_start(wfull[:], w_full[:])
            nc.sync.dma_start(msk[:], mask8[:])
            nc.sync.dma_start(ones[:], ones128[:])
            nc.sync.dma_start(onesA[:], ones_all[:])
            nc.sync.dma_start(idn16[:], ident16[:])
            nc.sync.dma_start(beh[:], b_eh[:])
            nc.sync.dma_start(emb[:], embT[:])

            with (
                tc.tile_pool(name="pre", bufs=1) as pf,
                tc.tile_pool(name="initw", bufs=5) as piw,
                tc.tile_pool(name="psum_pre", bufs=2, space="PSUM") as psp,
                tc.tile_pool(name="psum_init", bufs=1, space="PSUM") as psi,
            ):
                fW = pf.tile([128, CT, A], BF16)
                nc.sync.dma_start(fW[:], featWT[:])
                ftl = pf.tile([128, CT, NPF], BF16)
                nc.sync.dma_start(ftl[:], featT_loc[:])
                battn = pf.tile([128, 4], F32)
                nc.sync.dma_start(battn[:], b_attn[:])
                wihc = pf.tile([128, CT, G4], BF16)
                nc.sync.dma_start(wihc[:], w_ihcT[:])
                finitT = pf.tile([128, KT, N], BF16)
                nc.sync.dma_start(finitT[:], feat_initT[:])

                # initW stream (the DMA long pole): chunk DMAs issued before
                # the remaining gate weights so h0/c0 lands ASAP.
                iw_tiles = []
                for ch in range(NCHUNKS):
                    iw = piw.tile([128, NCH, 1024], BF16, tag="iw", name=f"iw{ch}")
                    nc.sync.dma_start(iw[:], initW[:, ch * NCH : (ch + 1) * NCH, :])
                    iw_tiles.append(iw)
                wie = pf.tile([128, 4, G4], BF16)
                for c4 in range(4):
                    nc.sync.dma_start(wie[:, c4, :], w_ieT[:, c4, :])
                for c4 in range(4):
                    nc.sync.dma_start(whh[:, c4, :], w_hhT[:, c4, :])

                # PE work, emission-interleaved so init consumes DMA chunks
                # while G/attn_img/gemb fill the gaps.
                ps_hc = psi.tile([64, 1024], F32)

                def init_chunk(ch):
                    iw = iw_tiles[ch]
                    for k in range(NCH):
                        kt = ch * NCH + k
                        for hf in range(2):
                            nc.tensor.matmul(
                                ps_hc[:, 512 * hf : 512 * (hf + 1)],
                                finitT[:, kt, 0:64],
                                iw[:, k, 512 * hf : 512 * (hf + 1)],
                                start=(kt == 0), stop=(kt == KT - 1),
                            )

                def attn_img_q(q):
                    ps_ai = psp.tile([128, NPF], F32, tag="pre", name=f"ai{q}")
                    for c in range(CT):
                        nc.tensor.matmul(
                            ps_ai[:], fW[:, c, 128 * q : 128 * (q + 1)],
                            ftl[:, c, :], start=(c == 0), stop=(c == CT - 1),
                        )
                    nc.scalar.activation(
                        attn_img[:, q, :], ps_ai[:], ACT.Identity,
                        bias=battn[:, q : q + 1],
                    )

                def g_piece(ch, gp):
                    ps_g = psp.tile([128, 256], F32, tag="pre", name=f"g{ch}_{gp}")
                    for c in range(CT):
                        nc.tensor.matmul(
                            ps_g[:],
                            ftl[:, c, 128 * ch : 128 * (ch + 1)],
                            wihc[:, c, 256 * gp : 256 * (gp + 1)],
                            start=(c == 0), stop=(c == CT - 1),
                        )
                    if gp % 2 == 0:
                        nc.vector.tensor_copy(
                            G_sb[:, ch, 256 * gp : 256 * (gp + 1)], ps_g[:]
                        )
                    else:
                        nc.scalar.activation(
                            G_sb[:, ch, 256 * gp : 256 * (gp + 1)], ps_g[:], ACT.Copy,
                        )

                def gemb_piece(gp):
                    ps_e = psp.tile([128, 2, T * NB], F32, tag="pre", name=f"ge{gp}")
                    for g2 in range(2):
                        k = 2 * gp + g2
                        for c in range(4):
                            nc.tensor.matmul(
                                ps_e[:, g2, :],
                                wie[:, c, 128 * k : 128 * (k + 1)],
                                emb[:, c, :, :].rearrange("e t n -> e (t n)"),
                                start=(c == 0), stop=(c == 3),
                            )
                    b0, b1 = bass.broadcast_tensor_aps(
                        ps_e[:],
                        beh[:, 2 * gp : 2 * gp + 2].rearrange("g (k o) -> g k o", o=1),
                    )
                    nc.vector.tensor_tensor(
                        gemb[:, 2 * gp : 2 * gp + 2, :], b0, b1, op=OP.add
                    )

                # emission order matches expected DMA arrival: attn_img
                # (feat weights land first), a few G pieces, then init chunks
                # alternating with the rest of G; gemb last (wie lands after
                # the initW stream).
                for q in range(4):
                    attn_img_q(q)
                gl = [(ch, gp) for ch in range(4) for gp in range(8)]
                for i in range(5):
                    g_piece(*gl[i])
                gi = 5
                for ch in range(NCHUNKS):
                    init_chunk(ch)
                    if gi < len(gl):
                        g_piece(*gl[gi])
                        gi += 1
                while gi < len(gl):
                    g_piece(*gl[gi])
                    gi += 1
                for gp in range(8):
                    gemb_piece(gp)

                # evac partials, exchange, transpose to [hc, n]
                hc_sb = pf.tile([64, 1024], F32)
                nc.vector.tensor_copy(hc_sb[:], ps_hc[:])
                hcp_d = dram.tile([64, 1024], F32)
                hcl_d = dram.tile([NB, 1024], F32)
                nc.scalar.dma_start(hcp_d[:], hc_sb[:])
                nc.gpsimd.collective_compute(
                    "ReduceScatter", OP.add, replica_groups=[list(range(NC_))],
                    ins=[hcp_d[:]], outs=[hcl_d[:]],
                )
                hcl = pf.tile([NB, 1024], F32)
                nc.scalar.dma_start(hcl[:], hcl_d[:])
                hcl16 = pf.tile([NB, 1024], BF16)
                nc.vector.tensor_copy(hcl16[:], hcl[:])
                with tc.tile_pool(name="psum_t", bufs=1, space="PSUM") as pst_p:
                    ps_t = pst_p.tile([128, 8, NB], BF16)
                    for j in range(8):
                        nc.tensor.transpose(
                            ps_t[:, j, :], hcl16[:, 128 * j : 128 * (j + 1)],
                            idn16[0:NB, 0:NB],
                        )
                    nc.vector.tensor_copy(h0_sb[:], ps_t[:, 0:4, :])
                    nc.vector.tensor_copy(c_a[:], ps_t[:, 4:8, :])

            # ---------------- recurrence + interleaved fc ----------------
            with (
                tc.tile_pool(name="fcw", bufs=1) as pfc,
                tc.tile_pool(name="loop", bufs=2) as pl,
                tc.tile_pool(name="stage", bufs=2) as pst,
                tc.tile_pool(name="ps_small", bufs=1, space="PSUM") as ps_sm_p,
                tc.tile_pool(name="ps_cxeh", bufs=1, space="PSUM") as ps_ce_p,
                tc.tile_pool(name="ps_fc", bufs=6, space="PSUM") as ps_fc_p,
            ):
                fcW = pfc.tile([128, 4, VP], BF16)
                nc.sync.dma_start(fcW[:], fc_WT[:])
                fcb = pfc.tile([128, VT], F32)
                nc.sync.dma_start(fcb[:], fc_b[:])

                c_cur, c_nxt = c_a, c_b
                fc_stage = {}
                fc_ei = [0]
                tail_mode = [False]

                fc_ps = {}

                def emit_fc_mm(vt, half):
                    ps_f = ps_fc_p.tile([128, 16 * NB], F32, tag="f")
                    fc_ps[(vt, half)] = ps_f
                    for c in range(4):
                        nc.tensor.matmul(
                            ps_f[:],
                            fcW[:, c, 128 * vt : 128 * (vt + 1)],
                            h_hist[:, c, 16 * half : 16 * (half + 1), :].rearrange(
                                "h t n -> h (t n)"
                            ),
                            start=(c == 0), stop=(c == 3),
                        )

                def emit_fc_evac(vt, half):
                    ps_f = fc_ps.pop((vt, half))
                    g0 = (vt // VG) * VG
                    if vt == g0:
                        fc_stage[(g0, half)] = pst.tile(
                            [128, VG, 16 * NB], BF16, tag="st", name=f"st{g0}_{half}"
                        )
                    st = fc_stage[(g0, half)]
                    if tail_mode[0] and fc_ei[0] % 2 == 0:
                        f0, f1 = bass.broadcast_tensor_aps(
                            ps_f[:], fcb[:, vt : vt + 1]
                        )
                        nc.vector.tensor_tensor(st[:, vt - g0, :], f0, f1, op=OP.add)
                    else:
                        nc.scalar.activation(
                            st[:, vt - g0, :], ps_f[:], ACT.Identity,
                            bias=fcb[:, vt : vt + 1],
                        )
                    fc_ei[0] += 1
                    if vt == min(g0 + VG, VT) - 1:
                        gn = vt - g0 + 1
                        nc.sync.dma_start(
                            preds_out[
                                128 * g0 : 128 * (g0 + gn),
                                16 * half : 16 * (half + 1), :,
                            ].rearrange("(g v) t n -> v g (t n)", g=gn),
                            st[:, 0:gn, :],
                        )

                def emit_fc(vt, half):
                    emit_fc_mm(vt, half)
                    emit_fc_evac(vt, half)

                for t in range(T):
                    h_prev = h0_sb[:, :, :] if t == 0 else h_hist[:, :, t - 1, :]

                    # one bank hosts the three small PSUM results (their
                    # accumulation groups are at disjoint offsets)
                    ps_u = ps_sm_p.tile([128, 44], F32, tag="u")
                    ps_ah = ps_u[:, 0:32].rearrange("p (q n) -> p q n", n=NB)
                    ps_st = ps_u[:, 32:36].rearrange("p (c o) -> p c o", o=1)
                    ps_z = ps_u[:, 36:44]
                    ps_ce = ps_ce_p.tile([128, 2, GT, NB], F32, tag="ce")
                    ps_eh = ps_ce[:, 0]
                    ps_cx = ps_ce[:, 1]
                    for q in range(4):
                        for kt in range(4):
                            nc.tensor.matmul(
                                ps_ah[:, q, :],
                                tokW[:, kt, 128 * q : 128 * (q + 1)],
                                h_prev[:, kt, :],
                                start=(kt == 0), stop=(kt == 3),
                            )

                    # W_hh gates (ready at step start; fills PE while DVE works)
                    for k in range(GT):
                        for c in range(4):
                            nc.tensor.matmul(
                                ps_eh[:, k, :],
                                whh[:, c, 128 * k : 128 * (k + 1)],
                                h_prev[:, c, :],
                                start=(c == 0), stop=(c == 3),
                            )

                    # interleaved fc matmuls (half 0 over steps 16..31);
                    # their Act evacuations are emitted after exp below.
                    fc_list = []
                    if t >= 16:
                        lo = (VT * (t - 16)) // 16
                        hi = (VT * (t - 15)) // 16
                        fc_list = list(range(lo, hi))
                    for vt in fc_list:
                        emit_fc_mm(vt, 0)

                    # e = relu(attn_img + ah), free order (q, p, n)
                    ah = pl.tile([128, 4, NB], BF16, tag="ahsb")
                    nc.vector.tensor_copy(ah[:], ps_ah[:])
                    e_pre = pl.tile([128, 4, P, NB], BF16, tag="epre")
                    i0 = attn_img[:, :, :].rearrange("a q (p n) -> a q p n", n=NB)
                    i1 = ah[:, :, :].rearrange("a q (o n) -> a q o n", o=1)
                    i0b, i1b = bass.broadcast_tensor_aps(i0, i1)
                    nc.vector.tensor_tensor(e_pre[:], i0b, i1b, op=OP.add)
                    e_sb = pl.tile([128, 4, NPF], BF16, tag="e")
                    nc.vector.tensor_scalar_max(
                        e_sb[:], e_pre[:].rearrange("a q p n -> a q (p n)"), 0.0
                    )

                    # veh = ps_eh + gemb_t is off the critical chain; placed
                    # here it fills DVE idle time during scoresT/exp.
                    v_eh = pl.tile([128, GT, NB], F32, tag="veh")
                    nc.vector.tensor_tensor(
                        v_eh[:], ps_eh[:], gemb[:, :, NB * t : NB * (t + 1)], op=OP.add
                    )

                    # scoresT: [(p16, n) chunk, c] = e_chunk^T @ w_full
                    for ch in range(4):
                        for q in range(4):
                            nc.tensor.matmul(
                                ps_st[:, ch, :],
                                e_sb[:, q, 128 * ch : 128 * (ch + 1)],
                                wfull[:, q : q + 1],
                                start=(q == 0), stop=(q == 3),
                            )
                    exps = pl.tile([128, 4, 1], BF16, tag="exps")
                    nc.scalar.activation(exps[:], ps_st[:], ACT.Exp)
                    for vt in fc_list:
                        emit_fc_evac(vt, 0)

                    # alpha_m[(p16, n), ch, n'] = exp * mask
                    alpha_m = pl.tile([128, 4, NB], BF16, tag="alm")
                    am0, am1 = bass.broadcast_tensor_aps(
                        exps[:, :, :], msk[:].rearrange("r (o n) -> r o n", o=1)
                    )
                    nc.vector.tensor_tensor(alpha_m[:], am0, am1, op=OP.mult)

                    # Z row first (its result chain gates the evac); the
                    # all-ones stationary replicates Z across all partitions,
                    # so reciprocal lands pre-broadcast -- no gpsimd hop.
                    for ch in range(4):
                        nc.tensor.matmul(
                            ps_z[:], onesA[:], alpha_m[:, ch, :],
                            start=(ch == 0), stop=(ch == 3),
                        )
                    for k in range(GT):
                        for ch in range(4):
                            nc.tensor.matmul(
                                ps_cx[:, k, :],
                                G_sb[:, ch, 128 * k : 128 * (k + 1)],
                                alpha_m[:, ch, :],
                                start=(ch == 0), stop=(ch == 3),
                            )
                    rz_rep = pl.tile([128, NB], F32, tag="rzr")
                    nc.vector.reciprocal(rz_rep[:], ps_z[:])

                    # gates = ps_cx * rz + veh
                    t0 = pl.tile([128, GT, NB], F32, tag="t0")
                    c0b, c1b = bass.broadcast_tensor_aps(
                        ps_cx[:, :, :], rz_rep[:].rearrange("g (o n) -> g o n", o=1)
                    )
                    nc.vector.tensor_tensor(t0[:], c0b, c1b, op=OP.mult)
                    gates = pl.tile([128, GT, NB], F32, tag="gates")
                    nc.vector.tensor_tensor(gates[:], t0[:], v_eh[:], op=OP.add)

                    # pointwise LSTM; tiles: i 0:4, f 4:8, o 8:12, g 12:16.
                    # One tanh(x/2) covers sigmoid gates AND tanh(g) (g-gate
                    # weights are host-doubled). State conventions: h~ = 2h
                    # (tokW/whh/fcW host-halved), c~ = 2c (init_Wc doubled).
                    #   sg_f = 0.5 th_f + 0.5
                    #   u = sg_f * c~          (= 2 sigmoid(f) c)
                    #   v = (th_i + 1) * th_g  (= 2 sigmoid(i) tanh(g))
                    #   c~_new = u + v
                    #   tanh_c = tanh(0.5 c~_new)
                    #   h~ = (th_o + 1) * tanh_c
                    th = pl.tile([128, GT, NB], F32, tag="th")
                    nc.scalar.activation(th[:], gates[:], ACT.Tanh, scale=0.5)
                    u2 = pl.tile([128, 4, NB], F32, tag="u2")
                    nc.vector.scalar_tensor_tensor(
                        u2[:], th[:, 4:8, :], 1.0, c_cur[:], op0=OP.add, op1=OP.mult,
                    )
                    t2 = pl.tile([128, 4, NB], F32, tag="t2")
                    nc.vector.scalar_tensor_tensor(
                        t2[:], th[:, 0:4, :], 1.0, th[:, 12:16, :],
                        op0=OP.add, op1=OP.mult,
                    )
                    nc.vector.scalar_tensor_tensor(
                        c_nxt[:], u2[:], 0.5, t2[:], op0=OP.mult, op1=OP.add,
                    )
                    tc_t = pl.tile([128, 4, NB], F32, tag="tc")
                    nc.scalar.activation(tc_t[:], c_nxt[:], ACT.Tanh, scale=0.5)
                    nc.vector.scalar_tensor_tensor(
                        h_hist[:, :, t, :], th[:, 8:12, :], 1.0, tc_t[:],
                        op0=OP.add, op1=OP.mult,
                    )
                    c_cur, c_nxt = c_nxt, c_cur

                # tail: fc half 1 (evacs alternate DVE/Act; both idle now)
                tail_mode[0] = True
                for vt in range(VT):
                    emit_fc(vt, 1)

    nc.compile()
    return nc


def _prep(inputs):
    """Host-side layout prep (slicing/transposes/dtype casts only)."""
    f = np.ascontiguousarray(np.asarray(inputs["features"], np.float32))
    cap = np.asarray(inputs["captions"])
    embd_W = np.asarray(inputs["embd_W"], np.float32)
    tokW = np.asarray(inputs["attn_token_W"], np.float32)
    tokb = np.asarray(inputs["attn_token_b"], np.float32)
    featW = np.asarray(inputs["attn_feat_W"], np.float32)
    featb = np.asarray(inputs["attn_feat_b"], np.float32)
    wfull = np.asarray(inputs["attn_full_W"], np.float32)
    W_ih = np.asarray(inputs["W_ih"], np.float32)
    b_ih = np.asarray(inputs["b_ih"], np.float32)
    W_hh = np.asarray(inputs["W_hh"], np.float32)
    b_hh = np.asarray(inputs["b_hh"], np.float32)
    fc_W = np.asarray(inputs["fc_W"], np.float32)
    fc_b = np.asarray(inputs["fc_b"], np.float32)
    iWh = np.asarray(inputs["init_Wh"], np.float32)
    iWc = np.asarray(inputs["init_Wc"], np.float32)

    def t128(x, pdim):  # (pdim*128, rest...) -> [128, pdim, rest]
        return np.ascontiguousarray(
            x.reshape(pdim, 128, *x.shape[1:]).transpose(1, 0, *range(2, x.ndim + 1))
        )

    # gate reorder (i, f, g, o) -> (i, f, o, g)
    ro = np.concatenate([
        np.arange(0, H), np.arange(H, 2 * H),
        np.arange(3 * H, 4 * H), np.arange(2 * H, 3 * H),
    ])

    featWT = t128(featW.T.copy(), CT).astype(bf16)              # [128,12,512]
    # h is carried as 2h on device; c as 2c; the g-gate rows are doubled so a
    # single tanh(x/2) computes both sigmoid halves and tanh(g).
    gsc = np.concatenate([np.ones(3 * H), 2.0 * np.ones(H)]).astype(np.float32)
    tok_WT = t128(0.5 * tokW.T.copy(), 4).astype(bf16)          # [128,4,512]
    b_attn = np.ascontiguousarray((tokb + featb).reshape(4, 128).T).astype(np.float32)
    w_full8 = np.ascontiguousarray(wfull[0].reshape(4, 128).T).astype(bf16)
    w_ihcT = t128((W_ih[ro] * gsc[:, None])[:, :ENC].T.copy(), CT).astype(bf16)
    w_ieT = t128((W_ih[ro] * gsc[:, None])[:, ENC:].T.copy(), 4).astype(bf16)
    w_hhT = t128((W_hh[ro] * (0.5 * gsc)[:, None]).T.copy(), 4).astype(bf16)
    b_eh = np.ascontiguousarray(
        ((b_ih[ro] + b_hh[ro]) * gsc).reshape(GT, 128).T
    ).astype(np.float32)                                        # [128,16]
    fc_pad = np.zeros((VP, H), np.float32)
    fc_pad[:V] = 0.5 * fc_W
    fc_WT = t128(fc_pad.T.copy(), 4).astype(bf16)               # [128,4,10112]
    fcb_pad = np.zeros((VP,), np.float32)
    fcb_pad[:V] = fc_b
    fc_b8 = np.ascontiguousarray(fcb_pad.reshape(VT, 128).T).astype(np.float32)
    mask8 = (np.arange(128)[:, None] % NB == np.arange(NB)[None, :]).astype(bf16)
    ones128 = np.ones((128, 1), bf16)
    ones_all = np.ones((128, 128), bf16)
    ident16 = np.eye(128).astype(bf16)

    in_maps = []
    for k in range(NC_):
        fl = f[NB * k : NB * (k + 1)]                    # (8, 64, 1536)
        featT_loc = t128(
            np.ascontiguousarray(fl.transpose(2, 1, 0)).reshape(ENC, P * NB), CT
        ).astype(bf16)                                   # [128,12,512] (p-major)
        embT = t128(
            np.ascontiguousarray(
                embd_W[cap[NB * k : NB * (k + 1)]].transpose(2, 1, 0)
            ).reshape(E, T * NB), 4,
        ).reshape(128, 4, T, NB).astype(bf16)
        fpix = np.ascontiguousarray(f[:, NB * k : NB * (k + 1), :])  # (64, 8, 1536)
        feat_initT = t128(fpix.reshape(N, KS).T.copy(), KT).astype(bf16)
        initWc = t128(
            2.0 * np.concatenate(
                [iWh[KS * k : KS * (k + 1)], iWc[KS * k : KS * (k + 1)]], axis=1
            ), KT,
        ).astype(bf16)                                   # [128,96,1024] (2h, 2c)
        in_maps.append({
            "featWT": featWT, "featT_loc": featT_loc, "b_attn": b_attn,
            "tok_WT": tok_WT, "w_full": w_full8,
            "w_ihcT": w_ihcT, "w_ieT": w_ieT, "w_hhT": w_hhT, "b_eh": b_eh,
            "embT": embT, "fc_WT": fc_WT, "fc_b": fc_b8,
            "feat_initT": feat_initT, "initW": initWc,
            "mask8": mask8, "ones128": ones128, "ones_all": ones_all,
            "ident16": ident16,
        })
    return in_maps


def kernel(**inputs) -> np.ndarray:
    if "nc" not in _cache:
        _cache["nc"] = _build()
    nc = _cache["nc"]
    in_maps = _prep(inputs)
    res = run_bass_kernel_spmd(nc, in_maps, core_ids=list(range(NC_)), trace=False)
    parts = [
        np.asarray(r["preds"][:V]).astype(np.float32).transpose(2, 1, 0)
        for r in res.results
    ]
    return np.ascontiguousarray(np.concatenate(parts, axis=0))


if __name__ == "__main__":
    import time
    t0 = time.time()
    nc = _build()
    print(f"build+compile: {time.time()-t0:.1f}s")
    from concourse.timeline_sim import TimelineSim
    ts = TimelineSim(nc, trace=False)
    print(f"TimelineSim: {ts.simulate():.0f} ns")
